# revision 1
# baseline (speedup 1.0000x reference)
"""Bass/Trainium2 kernel for softmax-weighted pattern mixing.

Reference computation (N=16384 patterns, each a 128x128 f32 matrix; x a
128x128 f32 matrix, D=16384):
    sims[n] = <P[n], x> / (|P[n]| * |x|)      (cosine similarity)
    w = softmax(sims)
    out = (w @ P) / N                          (128x128)

Strategy: shard patterns along N across 8 NeuronCores (2048 rows/core).
Each core makes ONE streaming pass over its 128 MiB shard (memory-bound):
  - dots[n]  = sum_d P[n,d]*x[d]   -> DVE tensor_tensor_reduce (fused mul+sum)
  - nsq[n]   = sum_d P[n,d]^2      -> ScalarE activation(Square, accum_out)
  - u[n]     = exp(dots[n] * rsqrt(nsq[n]) * (1/|x|))   (exp is safe
               unnormalized: cosine sims are bounded by 1)
  - acc[d]  += sum_n u[n]*P[n,d]   -> TensorE matmuls (float32r, full rate),
               accumulated in PSUM across all 16 blocks of 128 patterns.
Host gathers per-core partial acc and u sums, then out = acc/(N*sum(u)).
rsqrt is computed as exp(-0.5*ln(.)) so ScalarE uses a single activation
table set (natural_log_exp_and_others) -- no per-block table reloads.
"""

import sys

if "/opt/trn_rl_repo" not in sys.path:
    sys.path.insert(0, "/opt/trn_rl_repo")

import numpy as np
import ml_dtypes

N_CORES = 8
N = 16384            # total patterns
D = 16384            # elements per pattern (128*128)
P = 128              # SBUF partitions = patterns per block
N_LOC = N // N_CORES # 2048 patterns per core
NB = N_LOC // P      # 16 blocks per core
DMA_CHUNK = 8192     # f32 elems per DMA (4 MiB per transfer)
ST_CHUNK = 4096      # f32 elems per stats op
MM_N = 512           # matmul free dim (one PSUM bank)
NSLICE = D // MM_N   # 32 d-slices
N_BANKS = 8

_CACHE = {}


def _build():
    import concourse.bacc as bacc
    import concourse.tile as tile
    from concourse import mybir

    AF = mybir.ActivationFunctionType
    ALU = mybir.AluOpType
    f32 = mybir.dt.float32
    bf16 = mybir.dt.bfloat16
    f32r = mybir.dt.float32r
    AX = mybir.AxisListType

    nc = bacc.Bacc("TRN2", target_bir_lowering=False)
    pat = nc.dram_tensor("pat", [N_LOC, D], f32r, kind="ExternalInput")
    xrep_d = nc.dram_tensor("xrep", [P, D], bf16, kind="ExternalInput")
    acc_out = nc.dram_tensor("acc", [P, N_BANKS * MM_N], f32, kind="ExternalOutput")
    u_out = nc.dram_tensor("ustats", [P, NB], f32, kind="ExternalOutput")

    with tile.TileContext(nc) as tc:
        with (
            tc.tile_pool(name="xp", bufs=1) as xp,
            tc.tile_pool(name="blk", bufs=2) as blkp,
            tc.tile_pool(name="scr", bufs=2) as scrp,
            tc.tile_pool(name="ascr", bufs=2) as ascrp,
            tc.tile_pool(name="small", bufs=2) as smp,
            tc.tile_pool(name="fixed", bufs=1) as fxp,
            tc.tile_pool(name="psum", bufs=1, space="PSUM") as psp,
        ):
            xrep = xp.tile([P, D], bf16, tag="xrep")
            nc.sync.dma_start(out=xrep[:, :], in_=xrep_d[:, :])

            # 1/|x| : every partition holds the full x, so the free-dim
            # square-accumulate gives |x|^2 on every partition.
            xnp = fxp.tile([P, D // ST_CHUNK], f32, tag="xnp")
            for j in range(D // ST_CHUNK):
                a = ascrp.tile([P, ST_CHUNK], bf16, tag="ascr")
                nc.scalar.activation(
                    out=a[:, :],
                    in_=xrep[:, j * ST_CHUNK:(j + 1) * ST_CHUNK],
                    func=AF.Square,
                    accum_out=xnp[:, j:j + 1],
                )
            xnsq = fxp.tile([P, 1], f32, tag="xnsq")
            nc.vector.tensor_reduce(
                out=xnsq[:, :], in_=xnp[:, :], axis=AX.X, op=ALU.add
            )
            xln = fxp.tile([P, 1], f32, tag="xln")
            nc.scalar.activation(out=xln[:, :], in_=xnsq[:, :], func=AF.Ln)
            rxn = fxp.tile([P, 1], f32, tag="rxn")
            nc.scalar.activation(out=rxn[:, :], in_=xln[:, :], func=AF.Exp, scale=-0.5)

            ones32 = fxp.tile([P, 32], f32, tag="ones32")
            nc.vector.memset(ones32[:, :], 1.0)
            u_all = fxp.tile([P, NB], f32, tag="u_all")

            # Band weight tiles for the fp32r weighted-sum matmuls: uband[j]
            # has u in columns 32j..32j+31 and zeros elsewhere, so a full
            # M=128 matmul deposits the slice into PSUM partitions 32j..32j+31
            # (fp32r matmuls must write PSUM starting at partition 0).
            zeros128 = fxp.tile([P, P], f32, tag="zeros128")
            nc.vector.memset(zeros128[:, :], 0.0)
            ubands = []
            for j in range(4):
                ub = fxp.tile([P, P], f32r, tag=f"uband{j}", name=f"uband{j}")
                nc.vector.tensor_copy(out=ub[:, :], in_=zeros128[:, :])
                ubands.append(ub)

            psum_banks = [
                psp.tile([P, MM_N], f32, tag=f"ps{q}", name=f"psum{q}")
                for q in range(N_BANKS)
            ]

            for b in range(NB):
                blk = blkp.tile([P, D], f32r, tag="blk")
                for h in range(D // DMA_CHUNK):
                    sl = slice(h * DMA_CHUNK, (h + 1) * DMA_CHUNK)
                    nc.sync.dma_start(
                        out=blk[:, sl], in_=pat[b * P:(b + 1) * P, sl]
                    )

                nchunk = D // ST_CHUNK
                dch = smp.tile([P, nchunk], f32, tag="dch")
                npr = smp.tile([P, nchunk], f32, tag="npr")
                for j in range(nchunk):
                    sl = slice(j * ST_CHUNK, (j + 1) * ST_CHUNK)
                    scr = scrp.tile([P, ST_CHUNK], bf16, tag="scr")
                    nc.vector.scalar_tensor_tensor(
                        out=scr[:, :],
                        in0=blk[:, sl].bitcast(f32),
                        scalar=1.0,
                        in1=xrep[:, sl],
                        op0=ALU.mult,
                        op1=ALU.mult,
                        accum_out=dch[:, j:j + 1],
                    )
                    a2 = ascrp.tile([P, ST_CHUNK], bf16, tag="ascr")
                    nc.scalar.activation(
                        out=a2[:, :], in_=blk[:, sl].bitcast(f32), func=AF.Square,
                        accum_out=npr[:, j:j + 1],
                    )

                nsq = smp.tile([P, 1], f32, tag="nsq")
                nc.vector.tensor_reduce(
                    out=nsq[:, :], in_=npr[:, :], axis=AX.X, op=ALU.add
                )
                dsum = smp.tile([P, 1], f32, tag="dsum")
                nc.vector.tensor_reduce(
                    out=dsum[:, :], in_=dch[:, :], axis=AX.X, op=ALU.add
                )
                lnn = smp.tile([P, 1], f32, tag="lnn")
                nc.scalar.activation(out=lnn[:, :], in_=nsq[:, :], func=AF.Ln)
                rpn = smp.tile([P, 1], f32, tag="rpn")
                nc.scalar.activation(out=rpn[:, :], in_=lnn[:, :], func=AF.Exp, scale=-0.5)
                t = smp.tile([P, 1], f32, tag="t")
                nc.vector.tensor_tensor(
                    out=t[:, :], in0=dsum[:, :], in1=rpn[:, :], op=ALU.mult
                )
                # u = exp(dots * rpn * rxn)
                nc.scalar.activation(
                    out=u_all[:, b:b + 1], in_=t[:, :], func=AF.Exp, scale=rxn[:, 0:1]
                )
                for j in range(4):
                    nc.vector.tensor_scalar(
                        out=ubands[j][:, 32 * j:32 * (j + 1)], in0=ones32[:, :],
                        scalar1=u_all[:, b:b + 1], scalar2=None, op0=ALU.mult,
                    )

                for j in range(4):
                    for q in range(N_BANKS):
                        s = 4 * q + j
                        nc.tensor.matmul(
                            psum_banks[q][:, :],
                            ubands[j][:, :],
                            blk[:, s * MM_N:(s + 1) * MM_N],
                            start=(b == 0 and j == 0),
                            stop=(b == NB - 1 and j == 3),
                        )

            for q in range(N_BANKS):
                osb = scrp.tile([P, MM_N], f32, tag="scr")
                nc.vector.tensor_copy(out=osb[:, :], in_=psum_banks[q][:, :])
                nc.sync.dma_start(
                    out=acc_out[:, q * MM_N:(q + 1) * MM_N], in_=osb[:, :]
                )
            nc.sync.dma_start(out=u_out[:, :], in_=u_all[:, :])

    nc.finalize()
    return nc


def _get_nc():
    if "nc" not in _CACHE:
        _CACHE["nc"] = _build()
    return _CACHE["nc"]


def kernel(x, patterns):
    from concourse.bass_utils import run_bass_kernel_spmd

    x = np.asarray(x, dtype=np.float32)
    patterns = np.asarray(patterns, dtype=np.float32)

    nc = _get_nc()

    xrep = np.ascontiguousarray(
        np.broadcast_to(x.reshape(1, D), (P, D))
    ).astype(ml_dtypes.bfloat16)
    pat2d = patterns.reshape(N, D)

    in_maps = []
    for i in range(N_CORES):
        in_maps.append({
            "pat": pat2d[i * N_LOC:(i + 1) * N_LOC],
            "xrep": xrep,
        })

    res = run_bass_kernel_spmd(nc, in_maps, core_ids=list(range(N_CORES)))

    acc_total = np.zeros(D, dtype=np.float64)
    z_total = 0.0
    for i in range(N_CORES):
        acc_full = res.results[i]["acc"]      # [128, 4096] f32
        ustats = res.results[i]["ustats"]     # [128, 16] f32
        z_total += float(ustats.astype(np.float64).sum())
        for q in range(N_BANKS):
            for j in range(4):
                s = 4 * q + j
                acc_total[s * MM_N:(s + 1) * MM_N] += acc_full[
                    32 * j, q * MM_N:(q + 1) * MM_N
                ].astype(np.float64)

    out = (acc_total / (z_total * N)).astype(np.float32)
    return out.reshape(128, 128)

